# revision 1
# baseline (speedup 1.0000x reference)
"""ClassAlignmentLoss kernel for 8 TRN2 NeuronCores (Bass/Tile).

Data-parallel over N per domain: each core takes a contiguous 8192-sample
shard of every domain, computes local per-class segment sums/counts (one-hot
matmul on TensorE), all-reduces the [C, D+1] partials across the 8 cores,
then computes the compactness term from an SBUF-resident fp16 copy of its
feature shard (D = -F + onehotT.T @ centers via two matmuls, squared and
row-accumulated on ScalarE).  Center-distance terms are finished on host
from the (tiny, replicated) all-reduced sums.
"""

import numpy as np

# Problem shape (hardcoded per contract).
N_DOM = 3
N = 65536
D = 256
C = 64
ALPHA, BETA, GAMA = 1.0, 1.0, 1.0
N_CORES = 8
NSH = N // N_CORES          # samples per core per domain
P = 128                     # partitions / tile height


def build(nsh=NSH, n_chunks=4):
    """Build + compile the SPMD Bass module. nsh = per-core samples/domain."""
    import concourse.bass as bass
    import concourse.bacc as bacc
    import concourse.mybir as mybir
    import concourse.tile as tile

    dt = mybir.dt
    Alu = mybir.AluOpType
    Act = mybir.ActivationFunctionType

    tiles = nsh // P                    # 128-sample tiles per domain
    assert nsh % P == 0 and tiles % n_chunks == 0
    tpc = tiles // n_chunks             # tiles per DMA chunk
    half = nsh // 2                     # onehotT built in 2 half-domain chunks

    nc = bacc.Bacc(
        "TRN2",
        target_bir_lowering=False,
        debug=False,
        num_devices=N_CORES,
    )

    feat = nc.dram_tensor("feat", [N_DOM, nsh, D], dt.float32, kind="ExternalInput")
    labels = nc.dram_tensor("labels", [N_DOM, nsh], dt.int32, kind="ExternalInput")
    out_sums = nc.dram_tensor(
        "out_sums", [N_DOM, C, D + 1], dt.float32, kind="ExternalOutput"
    )
    out_comp = nc.dram_tensor("out_comp", [N_DOM, 1], dt.float32, kind="ExternalOutput")

    rg = [list(range(N_CORES))]

    with tile.TileContext(nc) as tc:
        with (
            tc.tile_pool(name="persist", bufs=1) as pp,
            tc.tile_pool(name="lab", bufs=2) as labp,
            tc.tile_pool(name="oh", bufs=3) as ohp,
            tc.tile_pool(name="ohT", bufs=1) as ohTp,
            tc.tile_pool(name="cent", bufs=2) as centp,
            tc.tile_pool(name="sq", bufs=3) as sqp,
            tc.tile_pool(name="pseg", bufs=1, space="PSUM") as psegp,
            tc.tile_pool(name="plc", bufs=1, space="PSUM") as plcp,
            tc.tile_pool(name="pcnt", bufs=1, space="PSUM") as pcntp,
            tc.tile_pool(name="pd", bufs=3, space="PSUM") as pdp,
            tc.tile_pool(name="ptrash", bufs=1, space="PSUM") as ptrashp,
            tc.tile_pool(name="pq", bufs=1, space="PSUM") as pqp,
            tc.tile_pool(name="dram", bufs=1, space="DRAM") as dramp,
        ):
            # ---- constants -------------------------------------------------
            iota64_i = pp.tile([P, C], dt.int16, tag="iota64_i")
            nc.gpsimd.iota(iota64_i[:], pattern=[[1, C]], base=0, channel_multiplier=0)
            # fp16 copy: 16-bit in/out lets the per-tile one-hot build hit 4x mode
            iota64 = pp.tile([P, C], dt.float16, tag="iota64")
            nc.vector.tensor_copy(iota64[:], iota64_i[:])
            iota128 = pp.tile([P, P], dt.int16, tag="iota128")
            nc.gpsimd.iota(iota128[:], pattern=[[1, P]], base=0, channel_multiplier=0)
            pidx_i = pp.tile([P, 1], dt.int32, tag="pidx_i")
            nc.gpsimd.iota(pidx_i[:], pattern=[[0, 1]], base=0, channel_multiplier=1)
            pidx = pp.tile([P, 1], dt.float32, tag="pidx")
            nc.vector.tensor_copy(pidx[:], pidx_i[:])
            cidx_i = pp.tile([C, 1], dt.int32, tag="cidx_i")
            nc.gpsimd.iota(cidx_i[:], pattern=[[0, 1]], base=0, channel_multiplier=1)
            cidx = pp.tile([C, 1], dt.float32, tag="cidx")
            nc.vector.tensor_copy(cidx[:], cidx_i[:])
            # negated identity (fp16): -1 where col == row
            negI = pp.tile([P, P], dt.float16, tag="negI")
            nc.vector.tensor_scalar(
                negI[:], iota128[:], pidx[:], -1.0, Alu.is_equal, Alu.mult
            )
            posI = pp.tile([P, P], dt.float16, tag="posI")
            nc.vector.tensor_scalar(
                posI[:], iota128[:], pidx[:], None, Alu.is_equal
            )
            ones_col = pp.tile([P, 1], dt.float32, tag="ones_col")
            nc.vector.memset(ones_col[:], 1.0)
            ones16 = pp.tile([P, 1], dt.float16, tag="ones16")
            nc.vector.memset(ones16[:], 1.0)

            # ---- persistent state -----------------------------------------
            # fp16 feature cache: [128, N_DOM * tiles * D]
            f16 = pp.tile([P, N_DOM * tiles * D], dt.float16, tag="f16")
            # per-sample sum-of-squares accumulator columns (one per tile)
            qsum = pqp.tile([P, N_DOM * tiles], dt.float32, tag="qsum")
            s_loc = [
                pp.tile([C, D + 1], dt.float32, tag=f"sloc{d}", name=f"sloc{d}")
                for d in range(N_DOM)
            ]
            s_glob = [
                pp.tile([C, D + 1], dt.float32, tag=f"sglob{d}", name=f"sglob{d}")
                for d in range(N_DOM)
            ]
            cnt_tmp = pp.tile([C, 2], dt.float32, tag="cnt_tmp")

            ohT = [
                ohTp.tile([C, nsh], dt.float16, tag=f"ohT{d}", name=f"ohT{d}")
                for d in range(N_DOM)
            ]

            # ================= phase 1: stream + segment sums ==============
            for d in range(N_DOM):
                dbase = d * tiles * D

                # labels: contiguous rows [64, 128], transpose on PE
                lab_rows = labp.tile([tiles, P], dt.float16, tag="lab_rows")
                nc.gpsimd.dma_start(
                    lab_rows[:], labels[d].rearrange("(t p) -> t p", p=P)
                )
                plc = plcp.tile([P, tiles], dt.float16, tag="plc")
                nc.tensor.transpose(plc[:], lab_rows[:], posI[:tiles, :tiles])
                lab_col = labp.tile([P, tiles], dt.float32, tag="lab_col")
                nc.vector.tensor_copy(lab_col[:], plc[:])

                # ---- features: cast-DMA fp32 -> fp16 into SBUF cache ------
                for k in range(n_chunks):
                    src = feat[d, k * tpc * P:(k + 1) * tpc * P, :].rearrange(
                        "(t p) m -> p t m", p=P
                    )
                    dst = f16[
                        :, dbase + k * tpc * D: dbase + (k + 1) * tpc * D
                    ].rearrange("p (t m) -> p t m", m=D)
                    nc.gpsimd.dma_start(dst, src)

                # ---- pass 1: segment sums ---------------------------------
                pseg = psegp.tile([C, D], dt.float32, tag="pseg")
                pcnt = pcntp.tile([C, 1], dt.float32, tag="pcnt")
                for t in range(tiles):
                    oh = ohp.tile([P, C], dt.float16, tag="oh")
                    nc.vector.tensor_scalar(
                        oh[:], iota64[:], lab_col[:, t:t + 1], None, Alu.is_equal
                    )
                    nc.tensor.matmul(
                        pseg[:],
                        oh[:],
                        f16[:, dbase + t * D: dbase + (t + 1) * D],
                        start=(t == 0),
                        stop=(t == tiles - 1),
                    )
                    nc.tensor.matmul(
                        pcnt[:],
                        oh[:],
                        ones16[:],
                        start=(t == 0),
                        stop=(t == tiles - 1),
                    )

                # ---- transposed one-hot + counts (2 half-domain chunks) ---
                for h in range(2):
                    lab_row = labp.tile([1, half], dt.float16, tag="lab_row")
                    nc.gpsimd.dma_start(
                        lab_row[:], labels[d:d + 1, h * half:(h + 1) * half]
                    )
                    lab_bc = labp.tile([C, half], dt.float16, tag="lab_bc")
                    nc.gpsimd.partition_broadcast(lab_bc[:], lab_row[:])
                    nc.vector.tensor_scalar(
                        ohT[d][:, h * half:(h + 1) * half],
                        lab_bc[:],
                        cidx[:],
                        None,
                        Alu.is_equal,
                    )
                nc.scalar.copy(s_loc[d][:, D:D + 1], pcnt[:])
                nc.scalar.copy(s_loc[d][:, 0:D], pseg[:])

                # ---- all-reduce the [C, D+1] partials ---------------------
                cc_in = dramp.tile([C, D + 1], dt.float32, tag=f"cc_in{d}")
                cc_out = dramp.tile([C, D + 1], dt.float32, tag=f"cc_out{d}")
                nc.sync.dma_start(cc_in[:], s_loc[d][:])
                nc.gpsimd.collective_compute(
                    "AllReduce",
                    Alu.add,
                    replica_groups=rg,
                    ins=[cc_in.opt()],
                    outs=[cc_out.opt()],
                )
                nc.sync.dma_start(s_glob[d][:], cc_out[:])
                nc.sync.dma_start(out_sums[d], cc_out[:])

            # ================= phase 2: compactness ========================
            for d in range(N_DOM):
                dbase = d * tiles * D

                cnt_cl = centp.tile([C, 1], dt.float32, tag="cnt_cl")
                nc.vector.tensor_scalar_max(cnt_cl[:], s_glob[d][:, D:D + 1], 1.0)
                inv = centp.tile([C, 1], dt.float32, tag="inv")
                nc.vector.reciprocal(inv[:], cnt_cl[:])
                cent16 = centp.tile([C, D], dt.float16, tag="cent16")
                nc.vector.tensor_scalar(
                    cent16[:], s_glob[d][:, 0:D], inv[:], None, Alu.mult
                )

                # D = -F + onehotT.T @ centers; q += D^2
                for t in range(tiles):
                    pd_t = pdp.tile([P, D], dt.float32, tag="pd")
                    nc.tensor.matmul(
                        pd_t[:],
                        negI[:],
                        f16[:, dbase + t * D: dbase + (t + 1) * D],
                        start=True,
                        stop=False,
                    )
                    nc.tensor.matmul(
                        pd_t[:],
                        ohT[d][:, t * P:(t + 1) * P],
                        cent16[:],
                        start=False,
                        stop=True,
                    )
                    qcol = qsum[:, d * tiles + t: d * tiles + t + 1]
                    if t % 6 == 0:
                        trash = ptrashp.tile([P, D], dt.float32, tag="trash")
                        nc.scalar.activation(
                            trash[:], pd_t[:], Act.Square, accum_out=qcol
                        )
                    else:
                        # ACT squares into fp16 SBUF; DVE does the row-reduce
                        sq16 = sqp.tile([P, D], dt.float16, tag="sq16")
                        nc.scalar.activation(sq16[:], pd_t[:], Act.Square)
                        dum16 = sqp.tile([P, D], dt.float16, tag="dum16")
                        nc.vector.tensor_scalar(
                            dum16[:], sq16[:], 1.0, None,
                            Alu.mult, Alu.add, accum_out=qcol,
                        )

            # ---- finale: dist = sqrt(q); per-domain partial sums ----------
            dist = pp.tile([P, N_DOM * tiles], dt.float32, tag="dist")
            nc.scalar.activation(dist[:], qsum[:], Act.Sqrt)
            dsum = pp.tile([P, N_DOM], dt.float32, tag="dsum")
            for d in range(N_DOM):
                nc.vector.reduce_sum(
                    dsum[:, d:d + 1],
                    dist[:, d * tiles:(d + 1) * tiles],
                    axis=mybir.AxisListType.X,
                )
            pc_t = plcp.tile([N_DOM, 1], dt.float32, tag="plc")
            nc.tensor.matmul(pc_t[:], dsum[:], ones_col[:], start=True, stop=True)
            comp_sb = pp.tile([N_DOM, 1], dt.float32, tag="comp_sb")
            nc.vector.tensor_copy(comp_sb[:], pc_t[:])
            nc.sync.dma_start(out_comp[:, :], comp_sb[:])

    nc.compile()
    return nc


_CACHED = {}


def _get_nc(nsh=NSH, n_chunks=4):
    key = (nsh, n_chunks)
    if key not in _CACHED:
        _CACHED[key] = build(nsh, n_chunks)
    return _CACHED[key]


def finish_host(out_maps, n_total):
    """Combine per-core outputs into the scalar loss (numpy, float64)."""
    comp_sum = np.zeros(N_DOM, dtype=np.float64)
    for m in out_maps:
        comp_sum += m["out_comp"].reshape(-1).astype(np.float64)
    comp = comp_sum / n_total

    S = out_maps[0]["out_sums"].astype(np.float64)   # [N_DOM, C, D+1]
    sums, counts = S[:, :, :D], S[:, :, D]
    centers = sums / np.maximum(counts, 1.0)[:, :, None]

    sep = np.zeros(N_DOM, dtype=np.float64)
    for d in range(N_DOM):
        cd = centers[d]
        sq = ((cd[:, None, :] - cd[None, :, :]) ** 2).sum(-1)
        dist = np.sqrt(np.maximum(sq, 0.0))
        np.fill_diagonal(dist, 0.0)
        sep[d] = dist.sum() / (C * (C - 1))

    intra = (BETA * comp.sum() - ALPHA * sep.sum()) / N_DOM
    inter = 0.0
    n_pairs = 0
    for i in range(N_DOM):
        for j in range(i + 1, N_DOM):
            inter += np.sqrt(((centers[i] - centers[j]) ** 2).sum()) / C
            n_pairs += 1
    inter /= n_pairs
    return np.float32(GAMA * intra + inter)


def shard_inputs(features, labels, nsh):
    features = np.ascontiguousarray(np.asarray(features), dtype=np.float32)
    labels = np.ascontiguousarray(np.asarray(labels), dtype=np.int32)
    in_maps = []
    for c in range(N_CORES):
        in_maps.append({
            "feat": np.ascontiguousarray(features[:, c * nsh:(c + 1) * nsh, :]),
            "labels": np.ascontiguousarray(labels[:, c * nsh:(c + 1) * nsh]),
        })
    return in_maps


def kernel(features, labels):
    from concourse.bass_utils import run_bass_kernel_spmd

    nc = _get_nc()
    in_maps = shard_inputs(features, labels, NSH)
    res = run_bass_kernel_spmd(nc, in_maps, core_ids=list(range(N_CORES)))
    return finish_host(res.results, N)



# revision 13
# speedup vs baseline: 1.2118x; 1.2118x over previous
"""ClassAlignmentLoss kernel for 8 TRN2 NeuronCores (Bass/Tile), v2.

Data-parallel over N per domain.  Host pre-casts the feature shard to fp16
in the exact SBUF layout ([128, dom*tile*257], col 256 of each tile row is
a 1.0 used by the fused cross-term), plus fp8 one-hot matrices (oh for the
phase-1 segment-sum stationary, ohT for the phase-2 center-select
stationary), so the device DMA is a flat contiguous copy.

Phase 1: per-tile one-hot matmuls accumulate per-class sums in PSUM while
the DMA streams; ScalarE squares the B/C tiles' features into ||f||^2
columns (batched, 8 tiles per ACTIVATE).  One fp32 AllReduce of all three
domains' [64, 256] sums.  Phase 2 computes q_i = ||f_i - c_{l_i}||^2 three
ways to balance engines: A tiles build (c_sel - f) in PSUM (negI matmul +
ohT matmul) and square-reduce on ScalarE+VectorE; B tiles use a fused
vector tensor_tensor_reduce of F_aug * (-2c | ||c||^2) against PSUM; C
tiles do the same on GpSimd.  q is shipped to the host, which does
sqrt/mean and the tiny center-distance terms in float64.
"""

import numpy as np

# Problem shape (hardcoded per contract).
N_DOM = 3
N = 65536
D = 256
C = 64
N_CORES = 8
NSH = N // N_CORES           # 8192 samples per core per domain
P = 128
TILES = NSH // P             # 64 tiles per domain
TT = N_DOM * TILES           # 192 tiles total
DS = D + 1                   # 257: feature row + trailing 1.0
WIN = 16                     # tiles per phase-2 window
NWIN = TT // WIN
ASUB = 8                     # tiles per A-subgroup (one PSUM tile)

# Window schedule: 'A' = all 16 tiles via ACT-square pipeline, 'M' = first
# 8 via ACT, last 8 via the fused DVE cross-term (those need ||f||^2,
# which GpSimd computes during phase 1).
WSCHED = "AAMAAMAAMAAM"

ALPHA, BETA, GAMA = 1.0, 1.0, 1.0


def _b_tiles(wsched=WSCHED):
    """Global tile indices handled by the B (DVE cross-term) pipeline."""
    out = []
    for w, ty in enumerate(wsched):
        if ty == "M":
            out.extend(range(w * WIN + 8, w * WIN + 16))
    return out


def build(wsched=WSCHED, warmup_cc=False):
    import concourse.bass as bass
    import concourse.bacc as bacc
    import concourse.mybir as mybir
    import concourse.tile as tile

    dt = mybir.dt
    Alu = mybir.AluOpType
    Act = mybir.ActivationFunctionType

    nc = bacc.Bacc(
        "TRN2",
        target_bir_lowering=False,
        debug=False,
        num_devices=N_CORES,
    )

    feat = nc.dram_tensor("feat", [P, TT * DS], dt.float16, kind="ExternalInput")
    oh_in = nc.dram_tensor("oh", [P, TT * C], dt.float8e4, kind="ExternalInput")
    ohT_in = nc.dram_tensor("ohT", [C, N_DOM * NSH], dt.float8e4, kind="ExternalInput")
    negI_in = nc.dram_tensor("negI", [P, P], dt.float16, kind="ExternalInput")
    invc_in = nc.dram_tensor("invc", [C, 8], dt.float32, kind="ExternalInput")
    out_sums = nc.dram_tensor("out_sums", [C, N_DOM * D], dt.float32, kind="ExternalOutput")
    out_q = nc.dram_tensor("out_q", [P, TT], dt.float32, kind="ExternalOutput")

    rg = [list(range(N_CORES))]

    with tile.TileContext(nc) as tc:
        with (
            tc.tile_pool(name="persist", bufs=1) as pp,
            tc.tile_pool(name="cent", bufs=2) as centp,
            tc.tile_pool(name="sq", bufs=3) as sqp,
            tc.tile_pool(name="ttrb", bufs=2) as ttrbp,
            tc.tile_pool(name="ttrc", bufs=2) as ttrcp,
            tc.tile_pool(name="pseg", bufs=2, space="PSUM") as psegp,
            tc.tile_pool(name="pa", bufs=1, space="PSUM") as pap,
            tc.tile_pool(name="pbc", bufs=2, space="PSUM") as pbcp,
            tc.tile_pool(name="dram", bufs=1, space="DRAM") as dramp,
        ):
            # ---- persistent SBUF -----------------------------------------
            f16 = pp.tile([P, TT * DS], dt.float16, tag="f16")
            oh = pp.tile([P, TT * C], dt.float8e4, tag="oh")
            ohT = pp.tile([C, N_DOM * NSH], dt.float8e4, tag="ohT")
            negI = pp.tile([P, P], dt.float16, tag="negI")
            invc = pp.tile([C, 8], dt.float32, tag="invc")
            qf = pp.tile([P, TT], dt.float32, tag="qf")
            qx = pp.tile([P, TT], dt.float32, tag="qx")
            qq = pp.tile([P, TT], dt.float32, tag="qq")
            s_all = pp.tile([C, N_DOM * D], dt.float32, tag="s_all")
            s_glob = pp.tile([C, N_DOM * D], dt.float32, tag="s_glob")

            nc.vector.memset(qf[:], 0.0)

            # ---- optional collective warm-up -----------------------------
            if warmup_cc:
                wz = pp.tile([C, 1], dt.float32, tag="wz")
                nc.vector.memset(wz[:], 0.0)
                wi = dramp.tile([C, 1], dt.float32, tag="warm_in")
                wo = dramp.tile([C, 1], dt.float32, tag="warm_out")
                nc.sync.dma_start(wi[:], wz[:])
                nc.gpsimd.collective_compute(
                    "AllReduce", Alu.add, replica_groups=rg,
                    ins=[wi.opt()], outs=[wo.opt()],
                )

            # ---- input DMA (flat contiguous copies) ----------------------
            nc.sync.dma_start(negI[:], negI_in[:])
            nc.sync.dma_start(invc[:], invc_in[:])
            for d in range(N_DOM):
                ob = d * TILES * C
                nc.sync.dma_start(oh[:, ob:ob + TILES * C], oh_in[:, ob:ob + TILES * C])
                fb = d * TILES * DS
                nch = 4
                step = TILES * DS // nch
                for k in range(nch):
                    a = fb + k * step
                    nc.sync.dma_start(f16[:, a:a + step], feat[:, a:a + step])
            for d in range(N_DOM):
                tb = d * NSH
                nc.sync.dma_start(ohT[:, tb:tb + NSH], ohT_in[:, tb:tb + NSH])

            # ================= phase 1: segment sums + ||f||^2 ============
            b_starts = sorted(_b_tiles(wsched))[::8]
            for d in range(N_DOM):
                pseg = psegp.tile([C, D], dt.float32, tag="pseg")
                for t in range(TILES):
                    g = d * TILES + t
                    nc.tensor.matmul(
                        pseg[:],
                        oh[:, g * C:(g + 1) * C],
                        f16[:, g * DS:g * DS + D],
                        start=(t == 0),
                        stop=(t == TILES - 1),
                    )
                # ||f||^2 for this domain's B tiles (ACT+DVE, idle in ph.1)
                for g0 in b_starts:
                    if not (d * TILES <= g0 < (d + 1) * TILES):
                        continue
                    fa = f16[:, g0 * DS:(g0 + 8) * DS].rearrange(
                        "p (t m) -> p t m", m=DS
                    )[:, :, 0:D]
                    sq = sqp.tile([P, 8 * D], dt.float16, tag="sqa")
                    sq3 = sq[:].rearrange("p (t m) -> p t m", m=D)
                    nc.scalar.activation(sq3, fa, Act.Square)
                    nc.vector.reduce_sum(
                        qf[:, g0:g0 + 8], sq3, axis=mybir.AxisListType.X
                    )
                nc.scalar.copy(s_all[:, d * D:(d + 1) * D], pseg[:])

            # ---- one all-reduce for all domains --------------------------
            cc_in = dramp.tile([C, N_DOM * D], dt.float32, tag="cc_in")
            cc_out = dramp.tile([C, N_DOM * D], dt.float32, tag="cc_out")
            nc.sync.dma_start(cc_in[:], s_all[:])
            nc.gpsimd.collective_compute(
                "AllReduce", Alu.add, replica_groups=rg,
                ins=[cc_in.opt()], outs=[cc_out.opt()],
            )
            nc.sync.dma_start(s_glob[:], cc_out[:])
            nc.sync.dma_start(out_sums[:, :], cc_out[:])

            # ================= phase 2: q = ||f - c_l||^2 =================
            for d in range(N_DOM):
                # centers: cpos = s/cnt (fp16), caug = [-2c | ||c||^2]
                cpos = centp.tile([C, D], dt.float16, tag="cpos")
                nc.vector.tensor_scalar(
                    cpos[:], s_glob[:, d * D:(d + 1) * D],
                    invc[:, 2 * d:2 * d + 1], None, Alu.mult,
                )
                caug = centp.tile([C, DS], dt.float16, tag="caug")
                nc.vector.tensor_scalar(
                    caug[:, 0:D], s_glob[:, d * D:(d + 1) * D],
                    invc[:, 2 * d + 1:2 * d + 2], None, Alu.mult,
                )
                ctr = centp.tile([C, D], dt.float16, tag="ctrash")
                cnorm = centp.tile([C, 1], dt.float32, tag="cnorm")
                nc.vector.scalar_tensor_tensor(
                    ctr[:], cpos[:], 1.0, cpos[:],
                    Alu.mult, Alu.mult, accum_out=cnorm[:],
                )
                nc.vector.tensor_copy(caug[:, D:DS], cnorm[:])

                for w in range(TILES // WIN):
                    wg = d * (TILES // WIN) + w
                    w0 = d * TILES + w * WIN
                    nsub = 2 if wsched[wg] == "A" else 1
                    # --- A tiles: diff in PSUM, square on ACT -------------
                    for s in range(nsub):
                        pa = pap.tile([P, ASUB * D], dt.float32, tag="pa")
                        for j in range(ASUB):
                            g = w0 + s * ASUB + j
                            tloc = w * WIN + s * ASUB + j
                            nc.tensor.matmul(
                                pa[:, j * D:(j + 1) * D],
                                negI[:],
                                f16[:, g * DS:g * DS + D],
                                start=True, stop=False,
                            )
                            nc.tensor.matmul(
                                pa[:, j * D:(j + 1) * D],
                                ohT[:, d * NSH + tloc * P:
                                     d * NSH + (tloc + 1) * P],
                                cpos[:],
                                start=False, stop=True,
                            )
                        sqa = sqp.tile([P, ASUB * D], dt.float16, tag="sqa")
                        sqa3 = sqa[:].rearrange("p (t m) -> p t m", m=D)
                        nc.scalar.activation(sqa3, pa[:].rearrange(
                            "p (t m) -> p t m", m=D), Act.Square)
                        g0 = w0 + s * ASUB
                        nc.vector.reduce_sum(
                            qx[:, g0:g0 + ASUB], sqa3, axis=mybir.AxisListType.X
                        )
                    # --- B tiles: fused cross term on DVE -----------------
                    if nsub == 1:
                        for j in range(8):
                            g = w0 + 8 + j
                            tloc = w * WIN + 8 + j
                            pbc = pbcp.tile([P, DS], dt.float32, tag="pbc")
                            nc.tensor.matmul(
                                pbc[:],
                                ohT[:, d * NSH + tloc * P: d * NSH + (tloc + 1) * P],
                                caug[:],
                                start=True, stop=True,
                            )
                            tb = ttrbp.tile([P, DS], dt.float16, tag="tb")
                            nc.vector.scalar_tensor_tensor(
                                tb[:], f16[:, g * DS:(g + 1) * DS], 1.0, pbc[:],
                                Alu.mult, Alu.mult,
                                accum_out=qx[:, g:g + 1],
                            )

            # ---- finale: q = qf + qx -> DRAM -----------------------------
            nc.vector.tensor_tensor(qq[:], qf[:], qx[:], Alu.add)
            nc.sync.dma_start(out_q[:, :], qq[:])

    nc.compile()
    return nc


_CACHED = {}


def _get_nc(key=(WSCHED, False)):
    if key not in _CACHED:
        _CACHED[key] = build(*key)
    return _CACHED[key]


def shard_inputs(features, labels):
    """Host prep: fp16 SBUF-layout features, fp8 one-hots, inv counts."""
    import ml_dtypes

    f8 = ml_dtypes.float8_e4m3
    features = np.asarray(features, dtype=np.float32)
    labels = np.asarray(labels, dtype=np.int32)

    counts = np.stack(
        [np.bincount(labels[d], minlength=C) for d in range(N_DOM)]
    ).astype(np.float64)                                   # [N_DOM, C] global
    cnt = np.maximum(counts, 1.0)
    invc_full = np.zeros((C, 8), dtype=np.float32)
    for d in range(N_DOM):
        invc_full[:, 2 * d] = (1.0 / cnt[d]).astype(np.float32)
        invc_full[:, 2 * d + 1] = (-2.0 / cnt[d]).astype(np.float32)

    negI = (-np.eye(P)).astype(np.float16)

    in_maps = []
    for c in range(N_CORES):
        fl = features[:, c * NSH:(c + 1) * NSH, :]          # [3, 8192, 256]
        lb = labels[:, c * NSH:(c + 1) * NSH]               # [3, 8192]
        # features -> [128, dom, tile, 257] fp16 with trailing 1.0
        fc = np.ones((P, N_DOM, TILES, DS), dtype=np.float16)
        fc[:, :, :, 0:D] = (
            fl.astype(np.float16).reshape(N_DOM, TILES, P, D).transpose(2, 0, 1, 3)
        )
        # one-hot (uint8 0x38 is fp8e4 1.0): oh [128, dom*tile*64]
        lr = lb.reshape(N_DOM, TILES, P)
        oh_u8 = (lr[:, :, :, None] == np.arange(C)[None, None, None, :])
        oh_u8 = (oh_u8.astype(np.uint8) * 0x38).transpose(2, 0, 1, 3)
        ohT_u8 = (lb[:, None, :] == np.arange(C)[None, :, None])
        ohT_u8 = (ohT_u8.astype(np.uint8) * 0x38).transpose(1, 0, 2)
        in_maps.append({
            "feat": np.ascontiguousarray(fc.reshape(P, TT * DS)),
            "oh": np.ascontiguousarray(oh_u8.reshape(P, TT * C)).view(f8),
            "ohT": np.ascontiguousarray(ohT_u8.reshape(C, N_DOM * NSH)).view(f8),
            "negI": negI,
            "invc": invc_full,
        })
    return in_maps, counts


def finish_host(out_maps, counts):
    """Combine per-core outputs into the scalar loss (numpy, float64)."""
    comp = np.zeros(N_DOM, dtype=np.float64)
    for m in out_maps:
        q = m["out_q"].astype(np.float64).reshape(P, N_DOM, TILES)
        dist = np.sqrt(np.maximum(q, 0.0))
        comp += dist.sum(axis=(0, 2))
    comp /= N

    S = out_maps[0]["out_sums"].astype(np.float64)          # [C, 3*D]
    cnt = np.maximum(counts, 1.0)                           # [3, C]
    sep = np.zeros(N_DOM, dtype=np.float64)
    centers = np.zeros((N_DOM, C, D), dtype=np.float64)
    for d in range(N_DOM):
        centers[d] = S[:, d * D:(d + 1) * D] / cnt[d][:, None]
        cd = centers[d]
        sq = ((cd[:, None, :] - cd[None, :, :]) ** 2).sum(-1)
        dist = np.sqrt(np.maximum(sq, 0.0))
        np.fill_diagonal(dist, 0.0)
        sep[d] = dist.sum() / (C * (C - 1))

    intra = (BETA * comp.sum() - ALPHA * sep.sum()) / N_DOM
    inter = 0.0
    n_pairs = 0
    for i in range(N_DOM):
        for j in range(i + 1, N_DOM):
            inter += np.sqrt(((centers[i] - centers[j]) ** 2).sum()) / C
            n_pairs += 1
    inter /= n_pairs
    return np.float32(GAMA * intra + inter)


def kernel(features, labels):
    from concourse.bass_utils import run_bass_kernel_spmd

    nc = _get_nc()
    in_maps, counts = shard_inputs(features, labels)
    res = run_bass_kernel_spmd(nc, in_maps, core_ids=list(range(N_CORES)))
    return finish_host(res.results, counts)


# revision 15
# speedup vs baseline: 1.6171x; 1.3345x over previous
"""ClassAlignmentLoss kernel for 8 TRN2 NeuronCores (Bass/Tile), v2b.

Data-parallel over N per domain.  Host pre-casts the feature shard to fp8e4
in the exact SBUF layout ([128, dom*tile*257], col 256 of each tile row is
a 1.0 used by the fused cross-term), plus fp8 one-hot matrices (oh for the
phase-1 segment-sum stationary, ohT for the phase-2 center-select
stationary).  Device DMA is a flat contiguous copy.

Phase 1: per-tile one-hot matmuls accumulate per-class sums in PSUM while
the DMA streams; ScalarE+VectorE compute ||f||^2 for the B-half tiles.
Domain 0's sums all-reduce first (fp32); domains 1-2 follow in a second
all-reduce that overlaps domain 0's phase 2.  Phase 2 computes
q_i = ||f_i - c_{l_i}||^2 two ways to balance engines: A tiles build
(c_sel - f) in PSUM (negI + ohT matmuls) and square-accumulate per tile on
ScalarE; B tiles do a fused DVE scalar_tensor_tensor of F_aug against the
PSUM (-2c | ||c||^2) selection.  q ships to the host, which does
sqrt/mean and the tiny center-distance terms in float64.
"""

import numpy as np

# Problem shape (hardcoded per contract).
N_DOM = 3
N = 65536
D = 256
C = 64
N_CORES = 8
NSH = N // N_CORES           # 8192 samples per core per domain
P = 128
TILES = NSH // P             # 64 tiles per domain
TT = N_DOM * TILES           # 192 tiles total
DS = D + 1                   # 257: feature row + trailing 1.0
WIN = 16                     # tiles per phase-2 window
ASUB = 4                     # tiles per A PSUM tile

ALPHA, BETA, GAMA = 1.0, 1.0, 1.0


def build(warmup_cc=False):
    import concourse.bass as bass
    import concourse.bacc as bacc
    import concourse.mybir as mybir
    import concourse.tile as tile

    dt = mybir.dt
    Alu = mybir.AluOpType
    Act = mybir.ActivationFunctionType

    nc = bacc.Bacc(
        "TRN2",
        target_bir_lowering=False,
        debug=False,
        num_devices=N_CORES,
    )

    feat = nc.dram_tensor("feat", [P, TT * DS], dt.float8e4, kind="ExternalInput")
    oh_in = nc.dram_tensor("oh", [P, TT * C], dt.float8e4, kind="ExternalInput")
    ohT_in = nc.dram_tensor("ohT", [C, N_DOM * NSH], dt.float8e4, kind="ExternalInput")
    negI_in = nc.dram_tensor("negI", [P, P], dt.float8e4, kind="ExternalInput")
    invc_in = nc.dram_tensor("invc", [C, 8], dt.float32, kind="ExternalInput")
    out_sums = nc.dram_tensor("out_sums", [C, N_DOM * D], dt.float32, kind="ExternalOutput")
    out_q = nc.dram_tensor("out_q", [P, TT], dt.float32, kind="ExternalOutput")

    rg = [list(range(N_CORES))]

    with tile.TileContext(nc) as tc:
        with (
            tc.tile_pool(name="persist", bufs=1) as pp,
            tc.tile_pool(name="cent", bufs=2) as centp,
            tc.tile_pool(name="sq", bufs=3) as sqp,
            tc.tile_pool(name="trA", bufs=3) as trap,
            tc.tile_pool(name="trB", bufs=3) as trbp,
            tc.tile_pool(name="pseg", bufs=1, space="PSUM") as psegp,
            tc.tile_pool(name="pa", bufs=2, space="PSUM") as pap,
            tc.tile_pool(name="pbc", bufs=2, space="PSUM") as pbcp,
            tc.tile_pool(name="dram", bufs=1, space="DRAM") as dramp,
        ):
            # ---- persistent SBUF -----------------------------------------
            f8 = pp.tile([P, TT * DS], dt.float8e4, tag="f8")
            oh = pp.tile([P, TT * C], dt.float8e4, tag="oh")
            ohT = pp.tile([C, N_DOM * NSH], dt.float8e4, tag="ohT")
            negI = pp.tile([P, P], dt.float8e4, tag="negI")
            invc = pp.tile([C, 8], dt.float32, tag="invc")
            qf = pp.tile([P, TT], dt.float32, tag="qf")
            qx = pp.tile([P, TT], dt.float32, tag="qx")
            qq = pp.tile([P, TT], dt.float32, tag="qq")
            s_all = pp.tile([C, N_DOM * D], dt.float32, tag="s_all")
            s_glob = pp.tile([C, N_DOM * D], dt.float32, tag="s_glob")

            nc.vector.memset(qf[:], 0.0)

            if warmup_cc:
                wz = pp.tile([C, 1], dt.float32, tag="wz")
                nc.vector.memset(wz[:], 0.0)
                wi = dramp.tile([C, 1], dt.float32, tag="warm_in")
                wo = dramp.tile([C, 1], dt.float32, tag="warm_out")
                nc.sync.dma_start(wi[:], wz[:])
                nc.gpsimd.collective_compute(
                    "AllReduce", Alu.add, replica_groups=rg,
                    ins=[wi.opt()], outs=[wo.opt()],
                )

            # ---- input DMA (flat contiguous copies) ----------------------
            nc.sync.dma_start(negI[:], negI_in[:])
            nc.sync.dma_start(invc[:], invc_in[:])
            for d in range(N_DOM):
                ob = d * TILES * C
                nc.sync.dma_start(oh[:, ob:ob + TILES * C], oh_in[:, ob:ob + TILES * C])
                fb = d * TILES * DS
                nch = 4
                step = TILES * DS // nch
                for k in range(nch):
                    a = fb + k * step
                    nc.sync.dma_start(f8[:, a:a + step], feat[:, a:a + step])
                tb = d * NSH
                nc.sync.dma_start(ohT[:, tb:tb + NSH], ohT_in[:, tb:tb + NSH])

            # ================= phase 1: segment sums + ||f||^2 ============
            cc_in = [None, None]
            cc_out = [None, None]
            for d in range(N_DOM):
                pseg = psegp.tile([C, D], dt.float32, tag="pseg")
                for t in range(TILES):
                    g = d * TILES + t
                    nc.tensor.matmul(
                        pseg[:],
                        oh[:, g * C:(g + 1) * C],
                        f8[:, g * DS:g * DS + D],
                        start=(t == 0),
                        stop=(t == TILES - 1),
                    )
                # ||f||^2 for the B half (last 8 of each 16) of this domain
                for w in range(TILES // WIN):
                    g0 = d * TILES + w * WIN + 8
                    fa = f8[:, g0 * DS:(g0 + 8) * DS].rearrange(
                        "p (t m) -> p t m", m=DS
                    )[:, :, 0:D]
                    sq = sqp.tile([P, 8 * D], dt.float16, tag="sqf")
                    sq3 = sq[:].rearrange("p (t m) -> p t m", m=D)
                    nc.scalar.activation(sq3, fa, Act.Square)
                    nc.vector.reduce_sum(
                        qf[:, g0:g0 + 8], sq3, axis=mybir.AxisListType.X
                    )
                nc.scalar.copy(s_all[:, d * D:(d + 1) * D], pseg[:])
                # split all-reduce: d0 alone, then d1+d2 together
                if d == 0:
                    cc_in[0] = dramp.tile([C, D], dt.float32, tag="cc_in0", name="cc_in0")
                    cc_out[0] = dramp.tile([C, D], dt.float32, tag="cc_out0", name="cc_out0")
                    nc.sync.dma_start(cc_in[0][:], s_all[:, 0:D])
                    nc.gpsimd.collective_compute(
                        "AllReduce", Alu.add, replica_groups=rg,
                        ins=[cc_in[0].opt()], outs=[cc_out[0].opt()],
                    )
                    nc.sync.dma_start(s_glob[:, 0:D], cc_out[0][:])
                    nc.sync.dma_start(out_sums[:, 0:D], cc_out[0][:])
                elif d == 2:
                    cc_in[1] = dramp.tile([C, 2 * D], dt.float32, tag="cc_in1", name="cc_in1")
                    cc_out[1] = dramp.tile([C, 2 * D], dt.float32, tag="cc_out1", name="cc_out1")
                    nc.sync.dma_start(cc_in[1][:], s_all[:, D:3 * D])
                    nc.gpsimd.collective_compute(
                        "AllReduce", Alu.add, replica_groups=rg,
                        ins=[cc_in[1].opt()], outs=[cc_out[1].opt()],
                    )
                    nc.sync.dma_start(s_glob[:, D:3 * D], cc_out[1][:])
                    nc.sync.dma_start(out_sums[:, D:3 * D], cc_out[1][:])

            # ================= phase 2: q = ||f - c_l||^2 =================
            for d in range(N_DOM):
                # centers: cpos = s/cnt (fp16), caug = [-2c | ||c||^2]
                cpos = centp.tile([C, D], dt.float16, tag="cpos")
                nc.vector.tensor_scalar(
                    cpos[:], s_glob[:, d * D:(d + 1) * D],
                    invc[:, 2 * d:2 * d + 1], None, Alu.mult,
                )
                caug = centp.tile([C, DS], dt.float16, tag="caug")
                nc.vector.tensor_scalar(
                    caug[:, 0:D], s_glob[:, d * D:(d + 1) * D],
                    invc[:, 2 * d + 1:2 * d + 2], None, Alu.mult,
                )
                ctr = centp.tile([C, D], dt.float16, tag="ctrash")
                cnorm = centp.tile([C, 1], dt.float32, tag="cnorm")
                nc.vector.scalar_tensor_tensor(
                    ctr[:], cpos[:], 1.0, cpos[:],
                    Alu.mult, Alu.mult, accum_out=cnorm[:],
                )
                nc.vector.tensor_copy(caug[:, D:DS], cnorm[:])

                for w in range(TILES // WIN):
                    w0 = d * TILES + w * WIN
                    # --- A half: diff in PSUM, per-tile square-accum ------
                    for s in range(8 // ASUB):
                        pa = pap.tile([P, ASUB * D], dt.float32, tag="pa")
                        for j in range(ASUB):
                            g = w0 + s * ASUB + j
                            tloc = w * WIN + s * ASUB + j
                            nc.tensor.matmul(
                                pa[:, j * D:(j + 1) * D],
                                negI[:],
                                f8[:, g * DS:g * DS + D],
                                start=True, stop=False,
                            )
                            nc.tensor.matmul(
                                pa[:, j * D:(j + 1) * D],
                                ohT[:, d * NSH + tloc * P:
                                     d * NSH + (tloc + 1) * P],
                                cpos[:],
                                start=False, stop=True,
                            )
                        for j in range(ASUB):
                            g = w0 + s * ASUB + j
                            ta = trap.tile([P, D], dt.float16, tag="ta")
                            nc.scalar.activation(
                                ta[:], pa[:, j * D:(j + 1) * D], Act.Square,
                                accum_out=qx[:, g:g + 1],
                            )
                    # --- B half: fused cross term on DVE ------------------
                    for j in range(8):
                        g = w0 + 8 + j
                        tloc = w * WIN + 8 + j
                        pbc = pbcp.tile([P, DS], dt.float32, tag="pbc")
                        nc.tensor.matmul(
                            pbc[:],
                            ohT[:, d * NSH + tloc * P: d * NSH + (tloc + 1) * P],
                            caug[:],
                            start=True, stop=True,
                        )
                        tb = trbp.tile([P, DS], dt.float16, tag="tb")
                        nc.vector.scalar_tensor_tensor(
                            tb[:], f8[:, g * DS:(g + 1) * DS], 1.0, pbc[:],
                            Alu.mult, Alu.mult,
                            accum_out=qx[:, g:g + 1],
                        )

            # ---- finale: q = qf + qx -> DRAM -----------------------------
            nc.vector.tensor_tensor(qq[:], qf[:], qx[:], Alu.add)
            nc.sync.dma_start(out_q[:, :], qq[:])

    nc.compile()
    return nc


_CACHED = {}


def _get_nc(key=(False,)):
    if key not in _CACHED:
        _CACHED[key] = build(*key)
    return _CACHED[key]


def shard_inputs(features, labels):
    """Host prep: fp8 SBUF-layout features + one-hots, inverse counts."""
    import ml_dtypes

    f8dt = ml_dtypes.float8_e4m3
    features = np.asarray(features, dtype=np.float32)
    labels = np.asarray(labels, dtype=np.int32)

    counts = np.stack(
        [np.bincount(labels[d], minlength=C) for d in range(N_DOM)]
    ).astype(np.float64)                                   # [N_DOM, C] global
    cnt = np.maximum(counts, 1.0)
    invc_full = np.zeros((C, 8), dtype=np.float32)
    for d in range(N_DOM):
        invc_full[:, 2 * d] = (1.0 / cnt[d]).astype(np.float32)
        invc_full[:, 2 * d + 1] = (-2.0 / cnt[d]).astype(np.float32)

    negI = (-np.eye(P)).astype(f8dt)

    in_maps = []
    for c in range(N_CORES):
        fl = features[:, c * NSH:(c + 1) * NSH, :]          # [3, 8192, 256]
        lb = labels[:, c * NSH:(c + 1) * NSH]               # [3, 8192]
        # features -> [128, dom, tile, 257] fp8 with trailing 1.0
        fc = np.ones((P, N_DOM, TILES, DS), dtype=f8dt)
        fc[:, :, :, 0:D] = (
            fl.reshape(N_DOM, TILES, P, D).transpose(2, 0, 1, 3)
        ).astype(f8dt)
        # one-hot (uint8 0x38 is fp8e4 1.0): oh [128, dom*tile*64]
        lr = lb.reshape(N_DOM, TILES, P)
        oh_u8 = (lr[:, :, :, None] == np.arange(C)[None, None, None, :])
        oh_u8 = (oh_u8.astype(np.uint8) * 0x38).transpose(2, 0, 1, 3)
        ohT_u8 = (lb[:, None, :] == np.arange(C)[None, :, None])
        ohT_u8 = (ohT_u8.astype(np.uint8) * 0x38).transpose(1, 0, 2)
        in_maps.append({
            "feat": np.ascontiguousarray(fc.reshape(P, TT * DS)),
            "oh": np.ascontiguousarray(oh_u8.reshape(P, TT * C)).view(f8dt),
            "ohT": np.ascontiguousarray(ohT_u8.reshape(C, N_DOM * NSH)).view(f8dt),
            "negI": negI,
            "invc": invc_full,
        })
    return in_maps, counts


def finish_host(out_maps, counts):
    """Combine per-core outputs into the scalar loss (numpy, float64)."""
    comp = np.zeros(N_DOM, dtype=np.float64)
    for m in out_maps:
        q = m["out_q"].astype(np.float64).reshape(P, N_DOM, TILES)
        dist = np.sqrt(np.maximum(q, 0.0))
        comp += dist.sum(axis=(0, 2))
    comp /= N

    S = out_maps[0]["out_sums"].astype(np.float64)          # [C, 3*D]
    cnt = np.maximum(counts, 1.0)                           # [3, C]
    sep = np.zeros(N_DOM, dtype=np.float64)
    centers = np.zeros((N_DOM, C, D), dtype=np.float64)
    for d in range(N_DOM):
        centers[d] = S[:, d * D:(d + 1) * D] / cnt[d][:, None]
        cd = centers[d]
        sq = ((cd[:, None, :] - cd[None, :, :]) ** 2).sum(-1)
        dist = np.sqrt(np.maximum(sq, 0.0))
        np.fill_diagonal(dist, 0.0)
        sep[d] = dist.sum() / (C * (C - 1))

    intra = (BETA * comp.sum() - ALPHA * sep.sum()) / N_DOM
    inter = 0.0
    n_pairs = 0
    for i in range(N_DOM):
        for j in range(i + 1, N_DOM):
            inter += np.sqrt(((centers[i] - centers[j]) ** 2).sum()) / C
            n_pairs += 1
    inter /= n_pairs
    return np.float32(GAMA * intra + inter)


def kernel(features, labels):
    from concourse.bass_utils import run_bass_kernel_spmd

    nc = _get_nc()
    in_maps, counts = shard_inputs(features, labels)
    res = run_bass_kernel_spmd(nc, in_maps, core_ids=list(range(N_CORES)))
    return finish_host(res.results, counts)
